# revision 8
# baseline (speedup 1.0000x reference)
"""DEMA (double exponential smoothing) Trainium2 Bass kernel.

Math
----
Reference recurrence (per batch b, channel c, over time t):
    s0 = x[0], b0 = x[1] - x[0]
    s_t = a*x_t + (1-a)*(s_{t-1} + b_{t-1})
    b_t = bt*(s_t - s_{t-1}) + (1-bt)*b_{t-1}
    out = [s0, s_1, ..., s_{T-1}]

Eliminating the trend state gives an exact 2nd-order recurrence
(s_0 = x_0, s_1 = x_1):
    s_t = tau*s_{t-1} - delta*s_{t-2} + b0*x_t + b1*x_{t-1},  t >= 2
    tau = 2 - a - a*bt, delta = 1 - a, b0 = a, b1 = a*((1-a)*(1+bt) - tau)

Blocked EXACT state-passing solution (any alpha/beta, real or complex
poles, no truncation): split time into blocks of L=95. Within a block,
out is linear in (the block's 95 x rows, the preceding x row, and the two
preceding outputs), so ONE 128x128 stationary weight per block does the
whole job on the TensorEngine:

    rhs partitions:  0      -> x_{start-1}
                     1..95  -> x_{start .. start+94}
                     96, 97 -> carry (out_{start-2}, out_{start-1})
                     98..127-> zero
    lhsT rows:       0 -> d_t, 1..95 -> Toeplitz w_{t-k+1},
                     96 -> psi_t, 97 -> phi_t          (host float64)
    lhsT cols:       0..94 -> out rows, 96,97 -> DUPLICATES of cols 93,94
                     (the next block's carry), rest zero

The duplicated columns make each matmul deposit the next block's carry at
PSUM partitions 96..127, so one quad-aligned ScalarE copy
(PSUM[96:128] -> next rhs[96:128]) closes the recurrence — no
cross-partition moves (engine APs must start at partition 0/32/64/96).
x_{start-1} rides along in the load DMA: block windows overlap by one row
and the affine DMA access pattern just re-reads it. Block 0 uses a
variant weight carrying the s_0=x_0 / s_1=x_1 initial-condition columns
(x at partitions 0..94, no carry); the ragged tail block (11 rows) uses a
truncated variant.

Per core: 44 blocks x 4 batches = 176 fp32 matmuls (~77 us on PE at
2 cyc/col), PSUM eviction split between VectorE/ScalarE, carries on
ScalarE — all under the ~190 us HBM roofline for 67 MB of traffic =>
memory bound. Sharding: batch 32 -> 4 per core over 8 cores (data
parallel; the recurrence is independent per (b, c)).
"""

import numpy as np

import concourse.bacc as bacc
import concourse.bass as bass
import concourse.mybir as mybir
from concourse import tile
from concourse.bass_utils import run_bass_kernel_spmd

N_CORES = 8
P = 128
B, T, C = 32, 4096, 512
BC = B // N_CORES   # batches per core
L = 95              # block length (partitions 96..127 reserved for carry)
MEGA = 10           # blocks per DMA mega-tile (950 rows ~ 1.9 MB)

_F32 = mybir.dt.float32


def _host_weight_pack(a: float, bt: float, t_len: int):
    """Three 128x128 lhsT weight blocks (first / middle / tail), fp32."""
    tau = 2.0 - a - a * bt
    delta = 1.0 - a
    b0 = a
    b1 = a * ((1.0 - a) * (1.0 + bt) - tau)
    n = L
    w = np.zeros(n)
    c0 = np.zeros(n)
    c1 = np.zeros(n)
    d = np.zeros(n)
    phi = np.zeros(n)
    psi = np.zeros(n)
    w[0] = b0
    w[1] = tau * b0 + b1
    c0[0] = 1.0
    c1[1] = 1.0
    d[0] = b1
    d[1] = tau * b1
    phi[0] = tau
    phi[1] = tau * tau - delta
    psi[0] = -delta
    psi[1] = -tau * delta
    for j in range(2, n):
        w[j] = tau * w[j - 1] - delta * w[j - 2]
        c0[j] = tau * c0[j - 1] - delta * c0[j - 2]
        c1[j] = tau * c1[j - 1] - delta * c1[j - 2] + (b1 if j == 2 else 0.0)
        d[j] = tau * d[j - 1] - delta * d[j - 2]
        phi[j] = tau * phi[j - 1] - delta * phi[j - 2]
        psi[j] = tau * psi[j - 1] - delta * psi[j - 2]
    tt = np.arange(L)[None, :]
    kk = np.arange(1, L + 1)[:, None]        # rhs partition 1..95 -> x off k-1
    lag = tt - (kk - 1)
    toe = np.where(lag >= 0, w[np.clip(lag, 0, n - 1)], 0.0)     # [95, 95]
    Wm = np.zeros((P, P))
    Wm[0, :L] = d
    Wm[1 : 1 + L, :L] = toe
    Wm[96, :L] = psi
    Wm[97, :L] = phi
    kk0 = np.arange(L)[:, None]              # block 0: partition k -> x_k
    lag0 = tt - kk0
    toe0 = np.where(lag0 >= 0, w[np.clip(lag0, 0, n - 1)], 0.0)
    W0 = np.zeros((P, P))
    W0[0, :L] = c0
    W0[1, :L] = c1
    W0[2:L, :L] = toe0[2:, :]
    for W in (Wm, W0):
        W[:, 96] = W[:, L - 2]
        W[:, 97] = W[:, L - 1]
    tl = t_len - (t_len // L) * L
    assert 0 < tl < L
    Wt = np.zeros((P, P))
    Wt[0, :tl] = d[:tl]
    Wt[1 : 1 + L, :tl] = toe[:, :tl]
    Wt[1 + tl : 1 + L, :] = 0.0              # x rows beyond the input are absent
    Wt[96, :tl] = psi[:tl]
    Wt[97, :tl] = phi[:tl]
    return np.stack([W0, Wm, Wt]).astype(np.float32)


def _build(bcount=BC, t_len=T, c_len=C):
    """Build + compile the per-core SPMD module (coefficient-independent)."""
    nb = t_len // L
    tl = t_len - nb * L
    assert 0 < tl
    megas = [(s, min(s + MEGA, nb)) for s in range(0, nb, MEGA)]
    nmega = len(megas)
    nc = bacc.Bacc("TRN2", target_bir_lowering=False, debug=False)
    x = nc.dram_tensor("x", [bcount, t_len, c_len], _F32, kind="ExternalInput")
    wd = nc.dram_tensor("wts", [3, P, P], _F32, kind="ExternalInput")
    y = nc.dram_tensor("y", [bcount, t_len, c_len], _F32, kind="ExternalOutput")

    with tile.TileContext(nc) as tc:
        with (
            tc.tile_pool(name="wpool", bufs=1) as wpool,
            tc.tile_pool(name="rpool", bufs=5) as rpool,
            tc.tile_pool(name="psum", bufs=6, space="PSUM") as pspool,
            tc.tile_pool(name="opool", bufs=2) as opool,
        ):
            wt = wpool.tile([P, 3 * P], _F32)
            nc.sync.dma_start(
                wt[:].rearrange("k (m t) -> k m t", m=3),
                wd[:].rearrange("m k t -> k m t"),
            )

            def wsl(idx):
                return wt[:, idx * P : (idx + 1) * P]

            for b in range(bcount):
                rts: dict = {}

                def load(m, b=b):
                    bs, be = megas[m]
                    nblk = be - bs
                    rt = rpool.tile([P, MEGA * c_len], _F32, tag="rt")
                    if m == 0:
                        # slot 0 fully zeroed (block 0 has no x_{-1}/carry;
                        # zero rows must meet zero weights, never garbage)
                        nc.gpsimd.memset(rt[:, 0:c_len], 0.0)
                        nc.sync.dma_start(rt[0:L, 0:c_len], x[b, 0:L, :])
                        if nblk > 1:
                            src = bass.AP(
                                x,
                                (b * t_len + L - 1) * c_len,
                                [[c_len, L + 1], [L * c_len, nblk - 1], [1, c_len]],
                            )
                            nc.sync.dma_start(
                                rt[0 : L + 1, c_len : nblk * c_len].rearrange(
                                    "p (blk c) -> p blk c", blk=nblk - 1
                                ),
                                src,
                            )
                    else:
                        src = bass.AP(
                            x,
                            (b * t_len + bs * L - 1) * c_len,
                            [[c_len, L + 1], [L * c_len, nblk], [1, c_len]],
                        )
                        nc.sync.dma_start(
                            rt[0 : L + 1, 0 : nblk * c_len].rearrange(
                                "p (blk c) -> p blk c", blk=nblk
                            ),
                            src,
                        )
                    return rt

                rts[0] = load(0)
                if nmega > 1:
                    rts[1] = load(1)
                rtt = rpool.tile([P, MEGA * c_len], _F32, tag="rt")
                nc.gpsimd.memset(rtt[:, 0:c_len], 0.0)
                nc.sync.dma_start(rtt[0 : tl + 1, 0:c_len], x[b, nb * L - 1 :, :])

                for m, (bs, be) in enumerate(megas):
                    if m + 2 < nmega:
                        rts[m + 2] = load(m + 2)
                    nblk = be - bs
                    ot = opool.tile([P, MEGA * c_len], _F32, tag="ot")
                    for s in range(nblk):
                        i = bs + s
                        ps = pspool.tile([P, c_len], _F32, tag="ps")
                        nc.tensor.matmul(
                            ps[:],
                            wsl(0 if i == 0 else 1),
                            rts[m][:, s * c_len : (s + 1) * c_len],
                            start=True,
                            stop=True,
                        )
                        # carry: PSUM top quad -> next block's rhs top quad
                        if s + 1 < nblk:
                            nxt = rts[m][:, (s + 1) * c_len : (s + 2) * c_len]
                        elif m + 1 < nmega:
                            nxt = rts[m + 1][:, 0:c_len]
                        else:
                            nxt = rtt[:, 0:c_len]
                        nc.scalar.copy(nxt[96:128, :], ps[96:128, :])
                        dst = ot[0:L, s * c_len : (s + 1) * c_len]
                        if i % 2 == 0:
                            nc.vector.tensor_copy(dst, ps[0:L, :])
                        else:
                            nc.scalar.copy(dst, ps[0:L, :])
                    ydst = y[b, bs * L : be * L, :].rearrange(
                        "(blk p) c -> p blk c", p=L
                    )
                    nc.scalar.dma_start(
                        ydst,
                        ot[0:L, 0 : nblk * c_len].rearrange(
                            "p (blk c) -> p blk c", blk=nblk
                        ),
                    )
                    del rts[m]
                # ragged tail block
                ps = pspool.tile([P, c_len], _F32, tag="ps")
                nc.tensor.matmul(ps[:], wsl(2), rtt[:, 0:c_len], start=True, stop=True)
                ott = opool.tile([P, MEGA * c_len], _F32, tag="ot")
                nc.vector.tensor_copy(ott[0:tl, 0:c_len], ps[0:tl, :])
                nc.scalar.dma_start(y[b, nb * L :, :], ott[0:tl, 0:c_len])
    nc.compile()
    return nc


_MODULE_CACHE: dict = {}


def _get_module(**kw):
    key = tuple(sorted(kw.items()))
    if key not in _MODULE_CACHE:
        _MODULE_CACHE[key] = _build(**kw)
    return _MODULE_CACHE[key]


def make_in_maps(x, alpha, beta, bcount=BC, t_len=T, n_cores=N_CORES):
    a = float(np.asarray(alpha).reshape(-1)[0])
    bt = float(np.asarray(beta).reshape(-1)[0])
    wts = _host_weight_pack(a, bt, t_len)
    in_maps = []
    for i in range(n_cores):
        xs = np.ascontiguousarray(x[i * bcount : (i + 1) * bcount], dtype=np.float32)
        in_maps.append({"x": xs, "wts": wts})
    return in_maps


def _run(x, alpha, beta, trace=False, **kw):
    x = np.asarray(x, dtype=np.float32)
    assert x.shape == (B, T, C), x.shape
    in_maps = make_in_maps(x, alpha, beta)
    nc = _get_module()
    res = run_bass_kernel_spmd(nc, in_maps, list(range(N_CORES)), trace=trace, **kw)
    out = np.concatenate([res.results[i]["y"] for i in range(N_CORES)], axis=0)
    return out, res


def kernel(x, alpha, beta):
    return _run(x, alpha, beta)[0]
